# revision 8
# baseline (speedup 1.0000x reference)
"""CrossViewTransformer kernel for 8 Trainium2 NeuronCores.

Math (per batch element b, n = H*W = 4096):
    q = wq @ xq + bq            [8, n]
    k = wk @ xr + bk            [8, n]
    v = wv @ xr + bv            [64, n]
    energy[j, i] = sum_p k[p, j] q[p, i]
    att = softmax(energy, axis=-1)          (softmax over i)
    z[c, j] = sum_i v[c, i] att[j, i]
    out = xq + z

Device strategy (data-parallel: one batch element per core):
  * Compute energy TRANSPOSED: eT[i, j] = sum_p q[p, i] k[p, j], tiled
    [128(i) x 1024(j)] in PSUM. In this layout softmax over i needs NO
    vector reductions at all: N[i, j] = exp(eT[i, j]) (ScalarE, PSUM->SBUF)
    and the denominator s[j] = sum_i N[i, j] falls out of the z matmul by
    augmenting v^T with a ones column:
        zu[., j] = [v^T | 1]^T @ N  ->  rows 0..63 = unnormalized z,
                                        row 64 = s[j].
    Max-subtraction is skipped: energies here are O(1) (|e| < ~6), exact
    softmax identity, fp32 exp is safe.
  * Energy and z matmuls run in bf16 (single-pass PE, fp32 PSUM accum);
    projections stay fp32.
  * PSUM: 4 banks = energy ping-pong (2 x [128,1024]), 4 banks = z accum
    ([65, 2048]); j processed in two 2048-wide blocks.
  * Biases are folded into the matmuls via a ones-row appended to the
    inputs (host-side) so they cost nothing.
  * q/k are produced replicated at partition strips 0/32/64/96 (via a
    host-side replicated weight layout) so the K=8 energy matmuls can be
    packed 4-per-PE-array with tile_position row tiling.
"""

import sys

if "/opt/trn_rl_repo" not in sys.path:
    sys.path.insert(0, "/opt/trn_rl_repo")

from contextlib import ExitStack

import numpy as np

import concourse.tile as tile
from concourse import bacc, mybir
from concourse.bass_utils import run_bass_kernel_spmd

B = 8
C = 64
HW = 4096
PROJ = 8
NCORES = 8

F32 = mybir.dt.float32
BF16 = mybir.dt.bfloat16
EXP = mybir.ActivationFunctionType.Exp
F16 = mybir.dt.float16

NT = HW // 128  # 32 i-tiles
JBW = 2048  # j block width (z psum = 4 banks)
NJB = HW // JBW  # 2
ECH = 1024  # energy chunk width (2 banks)
VTW = C + 1  # 65: v^T block width incl. ones column


def _build_nc():
    nc = bacc.Bacc("TRN2", target_bir_lowering=False, debug=False, num_devices=NCORES)

    xq_d = nc.dram_tensor("xq", [C + 1, HW], F32, kind="ExternalInput").ap()
    xr_d = nc.dram_tensor("xr", [C + 1, HW], F32, kind="ExternalInput").ap()
    wq_d = nc.dram_tensor("wq", [C + 1, 128], F32, kind="ExternalInput").ap()
    wk_d = nc.dram_tensor("wk", [C + 1, 128], F32, kind="ExternalInput").ap()
    wv_d = nc.dram_tensor("wv", [C + 1, VTW], F32, kind="ExternalInput").ap()
    out_d = nc.dram_tensor("out", [C, HW], F32, kind="ExternalOutput").ap()
    rs_d = nc.dram_tensor("rscratch", [NJB, JBW], F32).ap()

    with tile.TileContext(nc) as tc, ExitStack() as ctx:
        singles = ctx.enter_context(tc.tile_pool(name="singles", bufs=1))

        xq_sb = singles.tile([C + 1, HW], F32)
        xr_sb = singles.tile([C + 1, HW], F32)
        wq_sb = singles.tile([C + 1, 128], F32)
        wk_sb = singles.tile([C + 1, 128], F32)
        wv_sb = singles.tile([C + 1, VTW], F32)
        q_sb = singles.tile([128, HW], BF16)  # q replicated at strips 0/32/64/96
        k_sb = singles.tile([128, HW], BF16)
        vt_sb = singles.tile([128, NT * VTW], BF16)  # 32 blocks of [128, 65]

        xq_bf = singles.tile([C + 1, HW], BF16)
        xr_bf = singles.tile([C + 1, HW], BF16)
        wq_bf = singles.tile([C + 1, 128], BF16)
        wk_bf = singles.tile([C + 1, 128], BF16)
        wv_bf = singles.tile([C + 1, VTW], BF16)
        warm_sb = singles.tile([128, 512], BF16)

        # chunked input loads so the bf16 casts / projections start early
        NLC = 4  # load chunks
        LCW = HW // NLC
        for ci in range(NLC):
            nc.sync.dma_start(
                out=xq_sb[:, ci * LCW : (ci + 1) * LCW],
                in_=xq_d[:, ci * LCW : (ci + 1) * LCW],
            )
            nc.sync.dma_start(
                out=xr_sb[:, ci * LCW : (ci + 1) * LCW],
                in_=xr_d[:, ci * LCW : (ci + 1) * LCW],
            )
        nc.sync.dma_start(out=wq_sb[:, :], in_=wq_d[:, :])
        nc.sync.dma_start(out=wk_sb[:, :], in_=wk_d[:, :])
        nc.sync.dma_start(out=wv_sb[:, :], in_=wv_d[:, :])
        nc.vector.tensor_copy(out=wq_bf[:, :], in_=wq_sb[:, :])
        nc.vector.tensor_copy(out=wk_bf[:, :], in_=wk_sb[:, :])
        nc.vector.tensor_copy(out=wv_bf[:, :], in_=wv_sb[:, :])
        for ci in range(NLC):
            nc.vector.tensor_copy(
                out=xq_bf[:, ci * LCW : (ci + 1) * LCW],
                in_=xq_sb[:, ci * LCW : (ci + 1) * LCW],
            )
            nc.vector.tensor_copy(
                out=xr_bf[:, ci * LCW : (ci + 1) * LCW],
                in_=xr_sb[:, ci * LCW : (ci + 1) * LCW],
            )

        # ---- projections -------------------------------------------------
        with tc.tile_pool(name="setup_psum", bufs=2, space="PSUM") as sp:
            # PE warm-up: ~30 dependency-free back-to-back matmuls issued
            # while the input DMAs land, so the HAM clock gate opens
            # (1.2 -> 2.4 GHz) before the real work starts.
            nc.vector.memset(warm_sb[:, :], 0.0)
            wp = sp.tile([128, 512], F32, tag="warm")
            for _ in range(30):
                nc.tensor.matmul(
                    wp[:, :],
                    lhsT=warm_sb[:, 0:128],
                    rhs=warm_sb[:, :],
                    start=True,
                    stop=True,
                )
            for w_bf, x_bf, dst in ((wq_bf, xq_bf, q_sb), (wk_bf, xr_bf, k_sb)):
                for ci in range(HW // ECH):
                    pp = sp.tile([128, ECH], F32, tag="proj")
                    for h in range(ECH // 512):
                        nc.tensor.matmul(
                            pp[:, h * 512 : (h + 1) * 512],
                            lhsT=w_bf[:, :],
                            rhs=x_bf[
                                :, ci * ECH + h * 512 : ci * ECH + (h + 1) * 512
                            ],
                            start=True,
                            stop=True,
                        )
                    nc.vector.tensor_copy(
                        out=dst[:, ci * ECH : (ci + 1) * ECH], in_=pp[:, :]
                    )
            # v^T blocks: vt[i, c] = sum_ch xr_aug[ch, i] wv_aug[ch, c].
            # wv_aug carries a trailing unit column that turns xr_aug's ones
            # row into the ones column of each v^T block (the s[j] row of z).
            for t in range(NT):
                vp = sp.tile([128, VTW], F32, tag="vt")
                nc.tensor.matmul(
                    vp[:, :],
                    lhsT=xr_bf[:, t * 128 : (t + 1) * 128],
                    rhs=wv_bf[:, :],
                    start=True,
                    stop=True,
                )
                nc.vector.tensor_copy(
                    out=vt_sb[:, t * VTW : (t + 1) * VTW], in_=vp[:, :]
                )

        # ---- main loop ---------------------------------------------------
        # Per i-tile t: energy chunks [128, 1024] -> PSUM (2-way row-tiled
        # K=8 matmuls), DVE casts them to an fp16 SBUF staging buffer
        # (frees the PSUM ping fast), and ScalarE exps GRP i-tiles at a
        # time in one giant SBUF-sourced ACTIVATE (amortizes the ~352-cycle
        # per-instruction init that dominates PSUM-sourced exps).
        # z matmuls lag GRP tiles behind energy so the in-order PE never
        # waits on ScalarE.
        GRP = 4
        epool = ctx.enter_context(tc.tile_pool(name="epsum", bufs=2, space="PSUM"))
        zpool = ctx.enter_context(tc.tile_pool(name="zpsum", bufs=1, space="PSUM"))
        egpool = ctx.enter_context(tc.tile_pool(name="estage", bufs=2))
        ntpool = ctx.enter_context(tc.tile_pool(name="nt", bufs=2))
        fpool = ctx.enter_context(tc.tile_pool(name="fin", bufs=1))

        for jb in range(NJB):
            j0 = jb * JBW
            zps = zpool.tile([VTW, JBW], F32)

            ngrp = NT // GRP
            egs = [None] * ngrp
            nts = [None] * ngrp

            def emit_energy(t):
                g, ti = divmod(t, GRP)
                if ti == 0:
                    eg_t = egpool.tile([128, GRP * JBW], F16, tag="eg")
                    egs[g] = eg_t
                eg = egs[g]
                for e in range(JBW // ECH):
                    ep = epool.tile([128, ECH], F32, tag="e")
                    # HAM warmer: a throwaway matmul into the ping buffer
                    # right before its real refill. The PE would otherwise
                    # idle here (ScalarE is the bottleneck) and the activity
                    # monitor would re-throttle the array clock to 1.2 GHz.
                    nc.tensor.matmul(
                        ep[:, 0:256],
                        lhsT=warm_sb[:, 0:128],
                        rhs=warm_sb[:, 0:256],
                        start=True,
                        stop=True,
                    )
                    for h in range(ECH // 512):
                        strip = 32 * (2 * e + h)
                        jc = j0 + e * ECH + h * 512
                        nc.tensor.matmul(
                            ep[:, h * 512 : (h + 1) * 512],
                            lhsT=q_sb[
                                strip : strip + PROJ, t * 128 : (t + 1) * 128
                            ],
                            rhs=k_sb[strip : strip + PROJ, jc : jc + 512],
                            start=True,
                            stop=True,
                            tile_position=(strip, 0),
                        )
                    nc.vector.tensor_copy(
                        out=eg[:, ti * JBW + e * ECH : ti * JBW + (e + 1) * ECH],
                        in_=ep[:, :],
                    )

            def emit_exp(g):
                nt_g = ntpool.tile([128, GRP * JBW], BF16, tag="ntg")
                nts[g] = nt_g
                nc.scalar.activation(out=nt_g[:, :], in_=egs[g][:, :], func=EXP)

            def emit_z(t):
                g, ti = divmod(t, GRP)
                nt_g = nts[g]
                for c4 in range(JBW // 512):
                    nc.tensor.matmul(
                        zps[:, c4 * 512 : (c4 + 1) * 512],
                        lhsT=vt_sb[:, t * VTW : (t + 1) * VTW],
                        rhs=nt_g[:, ti * JBW + c4 * 512 : ti * JBW + (c4 + 1) * 512],
                        start=(t == 0),
                        stop=(t == NT - 1),
                    )

            for t in range(NT):
                emit_energy(t)
                if t % GRP == GRP - 1:
                    emit_exp(t // GRP)
                if t >= GRP:
                    emit_z(t - GRP)
            for t in range(NT - GRP, NT):
                emit_z(t)

            # ---- finalize: out = xq + z / s -----------------------------
            # Evacuate zu (and its s row) from PSUM, then compute 1/s at
            # full lane occupancy by spreading the s row over 128
            # partitions; broadcast r back over partitions via a DRAM
            # bounce (DMA partition-step-0 source is DRAM-only).
            z_sb = fpool.tile([VTW, JBW], F32, tag="z")
            nc.vector.tensor_copy(out=z_sb[:, :], in_=zps[:, :])
            ss_sb = fpool.tile([128, JBW // 128], F32, tag="ss")
            nc.sync.dma_start(out=ss_sb[:, :], in_=z_sb[C : C + 1, :])
            rr_sb = fpool.tile([128, JBW // 128], F32, tag="rr")
            nc.vector.reciprocal(out=rr_sb[:, :], in_=ss_sb[:, :])
            nc.sync.dma_start(out=rs_d[jb, :], in_=rr_sb[:, :])
            rb_sb = fpool.tile([C, JBW], F32, tag="rb")
            nc.sync.dma_start(
                out=rb_sb[:, :], in_=rs_d[jb : jb + 1, :].partition_broadcast(C)
            )
            o_sb = fpool.tile([C, JBW], F32, tag="o")
            nc.vector.tensor_mul(o_sb[:, :], z_sb[0:C, :], rb_sb[:, :])
            nc.vector.tensor_add(o_sb[:, :], o_sb[:, :], xq_sb[0:C, j0 : j0 + JBW])
            nc.sync.dma_start(out=out_d[:, j0 : j0 + JBW], in_=o_sb[:, :])

    nc.compile()
    return nc


_NC = None


def _get_nc():
    global _NC
    if _NC is None:
        _NC = _build_nc()
    return _NC


def _make_in_maps(query_x, ref_x, wq, bq, wk, bk, wv, bv):
    query_x = np.ascontiguousarray(np.asarray(query_x, dtype=np.float32))
    ref_x = np.ascontiguousarray(np.asarray(ref_x, dtype=np.float32))
    wq = np.asarray(wq, dtype=np.float32)
    bq = np.asarray(bq, dtype=np.float32)
    wk = np.asarray(wk, dtype=np.float32)
    bk = np.asarray(bk, dtype=np.float32)
    wv = np.asarray(wv, dtype=np.float32)
    bv = np.asarray(bv, dtype=np.float32)

    # weights replicated at partition strips (for energy row tiling), with
    # the bias as an extra contraction row (inputs carry a matching ones row)
    wq_rep = np.zeros((C + 1, 128), dtype=np.float32)
    wk_rep = np.zeros((C + 1, 128), dtype=np.float32)
    for r in range(4):
        wq_rep[:C, 32 * r : 32 * r + PROJ] = wq.T
        wq_rep[C, 32 * r : 32 * r + PROJ] = bq
        wk_rep[:C, 32 * r : 32 * r + PROJ] = wk.T
        wk_rep[C, 32 * r : 32 * r + PROJ] = bk
    wv_aug = np.zeros((C + 1, VTW), dtype=np.float32)
    wv_aug[:C, :C] = wv.T
    wv_aug[C, :C] = bv
    wv_aug[C, C] = 1.0  # unit column: xr_aug ones-row -> ones column of v^T

    ones = np.ones((1, HW), dtype=np.float32)
    in_maps = []
    for b in range(B):
        xq = np.concatenate([query_x[b].reshape(C, HW), ones], axis=0)
        xr = np.concatenate([ref_x[b].reshape(C, HW), ones], axis=0)
        in_maps.append(
            {
                "xq": np.ascontiguousarray(xq),
                "xr": np.ascontiguousarray(xr),
                "wq": wq_rep,
                "wk": wk_rep,
                "wv": wv_aug,
            }
        )
    return in_maps


def kernel(query_x, ref_x, wq, bq, wk, bk, wv, bv):
    nc = _get_nc()
    in_maps = _make_in_maps(query_x, ref_x, wq, bq, wk, bk, wv, bv)
    res = run_bass_kernel_spmd(nc, in_maps, core_ids=list(range(NCORES)))
    out = np.stack([r["out"].reshape(C, 64, 64) for r in res.results], axis=0)
    return np.ascontiguousarray(out.astype(np.float32))
